# revision 13
# baseline (speedup 1.0000x reference)
"""Causal single-head attention (shared-weight multi-head), 8-core Trainium2 Bass kernel.

Problem: embedded [4, 4096, 1024] f32, Wq/Wk/Wv [1024, 64] f32.
  q/k/v = embedded @ W*;  S = q k^T / 8 (causal);  P = softmax(S);  head = P v
  output = tile(head, 16) -> [4, 4096, 1024] f32.

Sharding: 8 cores = 4 batches x 2 roles; role 0 owns row blocks {0,3,4,7},
role 1 owns {1,2,5,6} (equal causal work). Host permutes the 8 token blocks
per core to positions (own0, oth0, own1, oth1, ...): slot s (= own block s,
rows at position 2s) attends key positions {0..2s+1}; diagonal at position
2s (static tri mask); position 2s+1 is valid-or-padding, zeroed via the exp
bias operand (role-specific data, -1e5 when padding).

Per-core pipeline (engines overlapped; proj and attention interleaved at
mm-group granularity so the PE FIFO never drains behind ACT):
  - x streamed per 512-token block: fp8e4 packed [128,4,2,T] for Q/K proj
    (DoubleRow matmuls, 2x), bf16 for V proj of positions 0-1 only.
  - Q^T emitted duplicated on both partition halves ([wq|wq] stationary);
    K^T emitted split even/odd chunks across partition halves -> the two
    128-key score chunks of a pair run as concurrent row-tiled matmuls
    (contraction 64 uses half the PE rows each).
  - V^T (fp8 DR + bf16 for pos 0/1) -> DMA-xbar transpose -> V1 = [V | 1]
    stationary tiles (fp8 pairs for DoubleRow PV, bf16 for slot 0).
  - scores pair -> one ACT exp per pair ([128,2,512] PSUM->SBUF, scale=1/8,
    bias=padbias) -> pt fp8/bf16 -> PV accumulates out^T[0:65,512] in PSUM
    (row 64 = softmax denominator via the ones column).
  - out^T staged to SBUF, DMA'd f32; host divides num/denom, un-permutes
    rows, tiles x16. x DMAs ride the SP HWDGE ring; transposes + output
    DMAs ride the ACT ring so neither blocks the other.
"""

import os
import numpy as np
import ml_dtypes

B, T, E, HEAD, NH = 4, 4096, 1024, 64, 16
BLK = 512
NCORES = 8
OWN = {0: [0, 3, 4, 7], 1: [1, 2, 5, 6]}
OTHER = {0: [1, 2, 5, 6], 1: [0, 3, 4, 7]}

_prog_cache = {}


def _build_program(reps=None):
    import concourse.bass as bass
    import concourse.mybir as mybir
    import concourse.tile as tile
    from concourse import bacc

    f32 = mybir.dt.float32
    bf16 = mybir.dt.bfloat16
    f8 = mybir.dt.float8e4
    DR = mybir.MatmulPerfMode.DoubleRow
    Exp = mybir.ActivationFunctionType.Exp

    nc = bacc.Bacc("TRN2", target_bir_lowering=False, debug=False, num_devices=NCORES)

    xf8 = nc.dram_tensor("xf8", [128, 8, 4, 2, BLK], f8, kind="ExternalInput").ap()
    xbf01 = nc.dram_tensor("xbf01", [128, 2, 8, BLK], bf16, kind="ExternalInput").ap()
    wq2 = nc.dram_tensor("wq2", [128, 4, 2, 128], f8, kind="ExternalInput").ap()
    wk2 = nc.dram_tensor("wk2", [128, 4, 2, 128], f8, kind="ExternalInput").ap()
    wvf8 = nc.dram_tensor("wvf8", [128, 4, 2, HEAD], f8, kind="ExternalInput").ap()
    wvbf = nc.dram_tensor("wvbf", [128, 8, HEAD], bf16, kind="ExternalInput").ap()
    wqbf = nc.dram_tensor("wqbf", [128, 8, 128], bf16, kind="ExternalInput").ap()
    wkbf = nc.dram_tensor("wkbf", [128, 8, 128], bf16, kind="ExternalInput").ap()
    trid = nc.dram_tensor("trid", [128, 128], bf16, kind="ExternalInput").ap()
    padb = nc.dram_tensor("padb", [128, 4], f32, kind="ExternalInput").ap()
    iden = nc.dram_tensor("iden", [128, 128], bf16, kind="ExternalInput").ap()
    out = nc.dram_tensor("out", [4, HEAD + 1, BLK], f32, kind="ExternalOutput").ap()

    import contextlib

    with tile.TileContext(nc) as tc:
        loop_ctx = tc.For_i(0, reps, 1) if reps else contextlib.nullcontext()
        with (
            loop_ctx,
            tc.tile_pool(name="singles", bufs=1) as singles,
            tc.tile_pool(name="stg", bufs=2) as stg,
            tc.tile_pool(name="ptp", bufs=6) as ptp,
            tc.tile_pool(name="psum_proj", bufs=2, space="PSUM") as psum_proj,
            tc.tile_pool(name="psum_s", bufs=2, space="PSUM") as psum_s,
            tc.tile_pool(name="psum_o", bufs=1, space="PSUM") as psum_o,
        ):
            # ---- persistent SBUF tiles (per-block where written in waves) ----
            x8w = [singles.tile([128, 4, 2, BLK], f8, name=f"x8w{b}")
                   for b in range(8)]
            xbw = [singles.tile([128, 8, BLK], bf16, name=f"xbw{b}")
                   for b in range(2)]
            wq_sb = singles.tile([128, 4, 2, 128], f8)
            wk_sb = singles.tile([128, 4, 2, 128], f8)
            wv8_sb = singles.tile([128, 4, 2, HEAD], f8)
            wvb_sb = singles.tile([128, 8, HEAD], bf16)
            wqb_sb = singles.tile([128, 8, 128], bf16)
            wkb_sb = singles.tile([128, 8, 128], bf16)
            tri_sb = singles.tile([128, 128], bf16)
            pb_sb = singles.tile([128, 4], f32)
            id_sb = singles.tile([128, 128], bf16)
            qts = [singles.tile([128, BLK], bf16, name=f"qt{s}") for s in range(4)]
            kts = [singles.tile([128, 2, 128], bf16, name=f"kt{b}")
                   for b in range(8)]
            v18 = [singles.tile([128, 2, 2, 80], f8, name=f"v18_{b}")
                   for b in range(8)]
            v1b = [singles.tile([128, 4, HEAD + 2], bf16, name=f"v1b{b}")
                   for b in range(2)]
            dummy = singles.tile([128, 1], f32)

            # ---- weights / masks / first waves; ACT table preload ----
            nc.sync.dma_start(out=wq_sb, in_=wq2)
            nc.sync.dma_start(out=wk_sb, in_=wk2)
            nc.sync.dma_start(out=wv8_sb, in_=wvf8)
            nc.sync.dma_start(out=wvb_sb, in_=wvbf)
            nc.sync.dma_start(out=wqb_sb, in_=wqbf)
            nc.sync.dma_start(out=wkb_sb, in_=wkbf)
            nc.sync.dma_start(out=tri_sb, in_=trid)
            nc.sync.dma_start(out=pb_sb, in_=padb)
            nc.sync.dma_start(out=id_sb, in_=iden)
            nc.sync.dma_start(out=x8w[0], in_=xf8[:, 0])
            nc.sync.dma_start(out=xbw[0], in_=xbf01[:, 0])
            nc.sync.dma_start(out=xbw[1], in_=xbf01[:, 1])
            nc.scalar.activation(dummy, pb_sb[:, 0:1], Exp)  # table load early
            for b in range(8):
                nc.vector.memset(v18[b][:, :, :, HEAD:HEAD + 1], 1.0)
                nc.vector.memset(v18[b][:, :, :, HEAD + 1:], 0.0)
            for b in range(2):
                nc.vector.memset(v1b[b][:, :, HEAD:HEAD + 1], 1.0)
                nc.vector.memset(v1b[b][:, :, HEAD + 1:], 0.0)

            # ---- emission thunks: proj groups / attention pairs ----
            def g_dma(b):
                def f():
                    nc.sync.dma_start(out=x8w[b], in_=xf8[:, b])
                return f

            def g_k(b):
                def f():
                    ps = psum_proj.tile([128, BLK], f32, tag="p", name=f"pk{b}")
                    if b == 0:
                        for k in range(8):
                            nc.tensor.matmul(ps, wkb_sb[:, k], xbw[0][:, k],
                                             start=(k == 0), stop=(k == 7))
                    else:
                        for t in range(4):
                            nc.tensor.matmul(ps, wk_sb[:, t], x8w[b][:, t],
                                             start=(t == 0), stop=(t == 3),
                                             perf_mode=DR)
                    src = ps.rearrange("p (t par f) -> p par t f", t=2, par=2)
                    nc.vector.tensor_copy(kts[b][0:64], src[0:64, 0])
                    nc.vector.tensor_copy(kts[b][64:128], src[64:128, 1])
                return f

            def g_q(s):
                def f():
                    ps = psum_proj.tile([128, BLK], f32, tag="p", name=f"pq{s}")
                    if s == 0:
                        for k in range(8):
                            nc.tensor.matmul(ps, wqb_sb[:, k], xbw[0][:, k],
                                             start=(k == 0), stop=(k == 7))
                    else:
                        for t in range(4):
                            nc.tensor.matmul(ps, wq_sb[:, t], x8w[2 * s][:, t],
                                             start=(t == 0), stop=(t == 3),
                                             perf_mode=DR)
                    nc.vector.tensor_copy(qts[s], ps)
                return f

            def g_v(b):
                def f():
                    ps = psum_proj.tile([128, BLK], f32, tag="p", name=f"pv{b}")
                    for t in range(4):
                        nc.tensor.matmul(ps[0:64, :], wv8_sb[:, t], x8w[b][:, t],
                                         start=(t == 0), stop=(t == 3),
                                         perf_mode=DR)
                    vts = stg.tile([128, BLK], bf16, tag="vts", name=f"vts{b}")
                    nc.vector.tensor_copy(vts[0:64, :], ps[0:64, :])
                    tp = psum_proj.tile([128, 4, HEAD], bf16, tag="tp",
                                        name=f"tp{b}", bufs=1)
                    for c in range(4):
                        nc.tensor.transpose(
                            tp[:, c], vts[0:64, c * 128:(c + 1) * 128],
                            id_sb[0:64, 0:HEAD])
                    nc.vector.tensor_copy(
                        v18[b][:, :, :, 0:HEAD],
                        tp.rearrange("p (t i) f -> p t i f", t=2))
                return f

            def g_vb(b):
                def f():
                    ps = psum_proj.tile([128, BLK], f32, tag="p", name=f"pvb{b}")
                    for k in range(8):
                        nc.tensor.matmul(ps[0:64, :], wvb_sb[:, k], xbw[b][:, k],
                                         start=(k == 0), stop=(k == 7))
                    vts = stg.tile([128, BLK], bf16, tag="vts", name=f"vtsb{b}")
                    nc.vector.tensor_copy(vts[0:64, :], ps[0:64, :])
                    tp = psum_proj.tile([128, 4, HEAD], bf16, tag="tp",
                                        name=f"tpb{b}", bufs=1)
                    for c in range(4):
                        nc.tensor.transpose(
                            tp[:, c], vts[0:64, c * 128:(c + 1) * 128],
                            id_sb[0:64, 0:HEAD])
                    nc.vector.tensor_copy(v1b[b][:, :, 0:HEAD], tp)
                return f

            slot_state = {}

            def g_pair(s, b, t):
                def f():
                    diag1 = (b == 2 * s and t == 1)
                    lo = 256 if diag1 else 0
                    sp = psum_s.tile([128, 2, BLK], f32, tag="s",
                                     name=f"s{s}b{b}t{t}")
                    for i in range(2):
                        nc.tensor.matmul(
                            sp[:, i, lo:BLK],
                            kts[b][64 * i:64 * i + 64, t],
                            qts[s][64 * i:64 * i + 64, lo:BLK],
                            start=True, stop=True)
                    bias = pb_sb[:, s:s + 1] if b == 2 * s + 1 else 0.0
                    dt = bf16 if s == 0 else f8
                    tag = "ptb" if s == 0 else "pt"
                    pt = ptp.tile([128, 2, BLK], dt, tag=tag,
                                  name=f"pt{s}b{b}t{t}")
                    nc.scalar.activation(pt[:, :, lo:BLK], sp[:, :, lo:BLK],
                                         Exp, bias=bias, scale=0.125)
                    if b == 2 * s:
                        c0 = 2 * t * 128
                        nc.vector.tensor_mul(
                            pt[:, 0, c0:c0 + 128], pt[:, 0, c0:c0 + 128], tri_sb)
                        nc.vector.tensor_mul(
                            pt[:, 1, c0 + 128:c0 + 256],
                            pt[:, 1, c0 + 128:c0 + 256], tri_sb)
                        nc.vector.memset(pt[:, 1, lo:lo + 128], 0.0)
                    oacc = slot_state[s]
                    first = slot_state.get((s, "first"), True)
                    last = (b == 2 * s + 1 and t == 1)
                    if s == 0:
                        for i in range(2):
                            nc.tensor.matmul(
                                oacc[0:HEAD + 2, lo:BLK], v1b[b][:, 2 * t + i],
                                pt[:, i, lo:BLK],
                                start=first and i == 0, stop=last and i == 1)
                    else:
                        nc.tensor.matmul(
                            oacc[0:80, lo:BLK], v18[b][:, t], pt[:, :, lo:BLK],
                            start=first, stop=last, perf_mode=DR)
                    slot_state[(s, "first")] = False
                    if last:
                        ost = stg.tile([128, BLK], f32, tag="ost", name=f"ost{s}")
                        nc.vector.tensor_copy(ost[0:HEAD + 1, :],
                                              oacc[0:HEAD + 1, :])
                        nc.sync.dma_start(out=out[s], in_=ost[0:HEAD + 1, :])
                return f

            # ---- interleaved emission schedule ----
            # Wave w's attention pairs reference only proj outputs emitted in
            # waves <= w, so they are held and interleaved with wave w+1's
            # proj groups (keeps the PE FIFO fed while ACT paces the pairs).
            prev_ap = []
            for w in range(8):
                pg = []
                if w < 7:
                    pg.append(g_dma(w + 1))
                pg.append(g_k(w))
                if w % 2 == 0:
                    pg.append(g_q(w // 2))
                pg.append(g_v(w))
                if w < 2:
                    pg.append(g_vb(w))
                while pg or prev_ap:
                    if pg:
                        pg.pop(0)()
                    if prev_ap:
                        prev_ap.pop(0)()
                if w % 2 == 0:
                    s = w // 2
                    slot_state[s] = psum_o.tile([128, BLK], f32, tag="o",
                                                name=f"o{s}")
                    slot_state[(s, "first")] = True
                    for b in range(2 * s + 1):
                        for t in range(2):
                            prev_ap.append(g_pair(s, b, t))
                else:
                    s = (w - 1) // 2
                    for t in range(2):
                        prev_ap.append(g_pair(s, 2 * s + 1, t))
            for f in prev_ap:
                f()

    nc.compile()
    return nc


def _host_inputs(embedded, Wq, Wk, Wv):
    """Per-core input maps (host does layout only: permute/pack/cast)."""
    bf = ml_dtypes.bfloat16
    f8 = ml_dtypes.float8_e4m3
    emb = np.asarray(embedded, dtype=np.float32)
    wq = np.asarray(Wq, dtype=np.float32)
    wk = np.asarray(Wk, dtype=np.float32)
    wv = np.asarray(Wv, dtype=np.float32)

    def packw(w, dup, dt):  # [1024, 64] -> [128, 4, 2, 64|128]
        r = w.reshape(4, 128, 2, HEAD).transpose(1, 0, 2, 3)
        if dup:
            r = np.concatenate([r, r], axis=3)
        return np.ascontiguousarray(r.astype(dt))

    wq2 = packw(wq, True, f8)
    wk2 = packw(wk, True, f8)
    wvf8 = packw(wv, False, f8)
    wvbf = np.ascontiguousarray(
        wv.reshape(8, 128, HEAD).transpose(1, 0, 2).astype(bf))

    def packwb(w):  # [1024, 64] -> [128, 8, 128] dup
        r = w.reshape(8, 128, HEAD).transpose(1, 0, 2)
        return np.ascontiguousarray(np.concatenate([r, r], axis=2).astype(bf))

    wqbf = packwb(wq)
    wkbf = packwb(wk)

    p = np.arange(128)[:, None]
    f = np.arange(128)[None, :]
    trid = np.ascontiguousarray((p <= f).astype(bf))
    idend = np.ascontiguousarray(np.eye(128).astype(bf))

    in_maps = []
    for b in range(B):
        for role in range(2):
            own, oth = OWN[role], OTHER[role]
            perm = [None] * 8
            for i in range(4):
                perm[2 * i] = own[i]
                perm[2 * i + 1] = oth[i]
            xT = emb[b].T  # [E, T]
            xp = np.concatenate(
                [xT[:, j * BLK:(j + 1) * BLK] for j in perm], axis=1)
            xf8 = np.ascontiguousarray(
                xp.reshape(4, 128, 2, 8, BLK)
                .transpose(1, 3, 0, 2, 4).astype(f8))
            xbf01 = np.ascontiguousarray(
                xp[:, 0:2 * BLK].reshape(8, 128, 2, BLK)
                .transpose(1, 2, 0, 3).astype(bf))
            pbv = np.where(np.array([oth[i] < own[i] for i in range(4)]),
                           0.0, -1e5).astype(np.float32)
            padb = np.ascontiguousarray(
                np.broadcast_to(pbv[None, :], (128, 4)).astype(np.float32))
            in_maps.append({
                "xf8": xf8, "xbf01": xbf01, "wq2": wq2, "wk2": wk2,
                "wvf8": wvf8, "wvbf": wvbf, "trid": trid, "padb": padb,
                "iden": idend, "wqbf": wqbf, "wkbf": wkbf,
            })
    return in_maps


def _run(nc, in_maps, trace=False):
    from concourse.bass_utils import run_bass_kernel_spmd
    return run_bass_kernel_spmd(nc, in_maps, list(range(NCORES)), trace=trace)


def _assemble(results):
    head = np.empty((B, T, HEAD), dtype=np.float32)
    for core, r in enumerate(results):
        b, role = divmod(core, 2)
        o = np.asarray(r["out"])  # [4, 65, 512]
        num = o[:, 0:HEAD, :]
        den = o[:, HEAD:HEAD + 1, :]
        h = (num / den).transpose(0, 2, 1)  # [4, 512, 64]
        for s in range(4):
            j = OWN[role][s]
            head[b, j * BLK:(j + 1) * BLK, :] = h[s]
    return np.tile(head, (1, 1, NH))


def kernel(embedded, Wq, Wk, Wv, num_heads):
    num_heads = int(num_heads)
    assert num_heads == NH

    if "nc" not in _prog_cache:
        _prog_cache["nc"] = _build_program()
    nc = _prog_cache["nc"]

    in_maps = _host_inputs(embedded, Wq, Wk, Wv)
    res = _run(nc, in_maps, trace=bool(int(os.environ.get("KERNEL_TRACE", "0"))))
    _prog_cache["last_result"] = res
    return _assemble(res.results)


# revision 20
# speedup vs baseline: 1.2880x; 1.2880x over previous
"""Causal single-head attention (shared-weight multi-head), 8-core Trainium2 Bass kernel.

Problem: embedded [4, 4096, 1024] f32, Wq/Wk/Wv [1024, 64] f32.
  q/k/v = embedded @ W*;  S = q k^T / 8 (causal);  P = softmax(S);  head = P v
  output = tile(head, 16) -> [4, 4096, 1024] f32.

Sharding: 8 cores = 4 batches x 2 roles; role 0 owns row blocks {0,3,4,7},
role 1 owns {1,2,5,6} (equal causal work). Host permutes the 8 token blocks
per core to positions (own0, oth0, own1, oth1, ...): slot s (= own block s,
rows at position 2s) attends key positions {0..2s+1}; diagonal at position
2s (static tri mask); position 2s+1 is valid-or-padding, zeroed via the exp
bias operand (role-specific data, -1e5 when padding).

Per-core pipeline (engines overlapped; proj and attention interleaved at
mm-group granularity so the PE FIFO never drains behind ACT):
  - x streamed per 512-token block: fp8e4 packed [128,4,2,T] for Q/K proj
    (DoubleRow matmuls, 2x), bf16 for V proj of positions 0-1 only.
  - Q^T emitted duplicated on both partition halves ([wq|wq] stationary);
    K^T emitted split even/odd chunks across partition halves -> the two
    128-key score chunks of a pair run as concurrent row-tiled matmuls
    (contraction 64 uses half the PE rows each).
  - V^T (fp8 DR + bf16 for pos 0/1) -> DMA-xbar transpose -> V1 = [V | 1]
    stationary tiles (fp8 pairs for DoubleRow PV, bf16 for slot 0).
  - scores pair -> one ACT exp per pair ([128,2,512] PSUM->SBUF, scale=1/8,
    bias=padbias) -> pt fp8/bf16 -> PV accumulates out^T[0:65,512] in PSUM
    (row 64 = softmax denominator via the ones column).
  - out^T staged to SBUF, DMA'd f32; host divides num/denom, un-permutes
    rows, tiles x16. x DMAs ride the SP HWDGE ring; transposes + output
    DMAs ride the ACT ring so neither blocks the other.
"""

import os
import numpy as np
import ml_dtypes

B, T, E, HEAD, NH = 4, 4096, 1024, 64, 16
BLK = 512
NCORES = 8
OWN = {0: [0, 3, 4, 7], 1: [1, 2, 5, 6]}
OTHER = {0: [1, 2, 5, 6], 1: [0, 3, 4, 7]}

_prog_cache = {}
DBG = []


def _build_program(reps=None):
    import concourse.bass as bass
    import concourse.mybir as mybir
    import concourse.tile as tile
    from concourse import bacc

    f32 = mybir.dt.float32
    bf16 = mybir.dt.bfloat16
    f8 = mybir.dt.float8e4
    DR = mybir.MatmulPerfMode.DoubleRow
    Exp = mybir.ActivationFunctionType.Exp

    nc = bacc.Bacc("TRN2", target_bir_lowering=False, debug=False, num_devices=NCORES)

    xf8 = nc.dram_tensor("xf8", [128, 8, 4, 2, BLK], f8, kind="ExternalInput").ap()
    xbf01 = nc.dram_tensor("xbf01", [128, 2, 8, BLK], bf16, kind="ExternalInput").ap()
    wq2 = nc.dram_tensor("wq2", [128, 4, 2, 128], f8, kind="ExternalInput").ap()
    wk2 = nc.dram_tensor("wk2", [128, 4, 2, 128], f8, kind="ExternalInput").ap()
    wvf8 = nc.dram_tensor("wvf8", [128, 4, 2, HEAD], f8, kind="ExternalInput").ap()
    wvbf = nc.dram_tensor("wvbf", [128, 8, HEAD], bf16, kind="ExternalInput").ap()
    wqbf = nc.dram_tensor("wqbf", [128, 8, 128], bf16, kind="ExternalInput").ap()
    wkbf = nc.dram_tensor("wkbf", [128, 8, 128], bf16, kind="ExternalInput").ap()
    trid = nc.dram_tensor("trid", [128, 128], bf16, kind="ExternalInput").ap()
    padb = nc.dram_tensor("padb", [128, 4], f32, kind="ExternalInput").ap()
    iden = nc.dram_tensor("iden", [128, 128], bf16, kind="ExternalInput").ap()
    out = nc.dram_tensor("out", [4, HEAD + 1, BLK], f32, kind="ExternalOutput").ap()

    import contextlib

    with tile.TileContext(nc) as tc:
        loop_ctx = tc.For_i(0, reps, 1) if reps else contextlib.nullcontext()
        with (
            loop_ctx,
            tc.tile_pool(name="singles", bufs=1) as singles,
            tc.tile_pool(name="stg", bufs=2) as stg,
            tc.tile_pool(name="ptp", bufs=6) as ptp,
            tc.tile_pool(name="psum_proj", bufs=2, space="PSUM") as psum_proj,
            tc.tile_pool(name="psum_s", bufs=2, space="PSUM") as psum_s,
            tc.tile_pool(name="psum_o", bufs=2, space="PSUM") as psum_o,
        ):
            # ---- persistent SBUF tiles (per-block where written in waves) ----
            x8w = [singles.tile([128, 4, 2, BLK], f8, name=f"x8w{b}")
                   for b in range(8)]
            xbw = [singles.tile([128, 8, BLK], bf16, name=f"xbw{b}")
                   for b in range(2)]
            wq_sb = singles.tile([128, 4, 2, 128], f8)
            wk_sb = singles.tile([128, 4, 2, 128], f8)
            wv8_sb = singles.tile([128, 4, 2, HEAD], f8)
            wvb_sb = singles.tile([128, 8, HEAD], bf16)
            wqb_sb = singles.tile([128, 8, 128], bf16)
            wkb_sb = singles.tile([128, 8, 128], bf16)
            tri_sb = singles.tile([128, 128], bf16)
            pb_sb = singles.tile([128, 4], f32)
            id_sb = singles.tile([128, 128], bf16)
            qts = [singles.tile([128, BLK], bf16, name=f"qt{s}") for s in range(4)]
            kts = [singles.tile([128, 2, 128], bf16, name=f"kt{b}")
                   for b in range(8)]
            v18 = [singles.tile([128, 2, 2, 80], f8, name=f"v18_{b}")
                   for b in range(8)]
            v1b = [singles.tile([128, 4, HEAD + 2], bf16, name=f"v1b{b}")
                   for b in range(2)]
            dummy = singles.tile([128, 1], f32)

            # ---- weights / masks / first waves; ACT table preload ----
            # critical-first: padb gates the ACT table preload; wkb/wqb
            # gate wave-0 projections; fp8 weights are needed a wave later.
            nc.scalar.dma_start(out=pb_sb, in_=padb)
            nc.scalar.dma_start(out=wkb_sb, in_=wkbf)
            nc.scalar.dma_start(out=wqb_sb, in_=wqbf)
            nc.scalar.dma_start(out=wvb_sb, in_=wvbf)
            nc.scalar.dma_start(out=wv8_sb, in_=wvf8)
            nc.scalar.dma_start(out=tri_sb, in_=trid)
            nc.scalar.dma_start(out=id_sb, in_=iden)
            nc.scalar.dma_start(out=wk_sb, in_=wk2)
            nc.scalar.dma_start(out=wq_sb, in_=wq2)
            # all x waves stream upfront in wave-arrival order (no deps)
            nc.sync.dma_start(out=xbw[0], in_=xbf01[:, 0])
            nc.sync.dma_start(out=x8w[0], in_=xf8[:, 0])
            nc.sync.dma_start(out=x8w[2], in_=xf8[:, 2])
            nc.sync.dma_start(out=xbw[1], in_=xbf01[:, 1])
            nc.sync.dma_start(out=x8w[1], in_=xf8[:, 1])
            for _b in (3, 4, 6, 5, 7):
                nc.sync.dma_start(out=x8w[_b], in_=xf8[:, _b])
            nc.scalar.activation(dummy, pb_sb[:, 0:1], Exp)  # table load early
            for b in range(8):
                nc.vector.memset(v18[b][:, :, :, HEAD:HEAD + 1], 1.0)
                nc.vector.memset(v18[b][:, :, :, HEAD + 1:], 0.0)
            for b in range(2):
                nc.vector.memset(v1b[b][:, :, HEAD:HEAD + 1], 1.0)
                nc.vector.memset(v1b[b][:, :, HEAD + 1:], 0.0)

            # ---- emission thunks: proj groups / attention pairs ----
            def g_k(b):
                def f():
                    ps = psum_proj.tile([128, BLK], f32, tag="p", name=f"pk{b}")
                    if b == 0:
                        for k in range(8):
                            nc.tensor.matmul(ps, wkb_sb[:, k], xbw[0][:, k],
                                             start=(k == 0), stop=(k == 7))
                    else:
                        for t in range(4):
                            nc.tensor.matmul(ps, wk_sb[:, t], x8w[b][:, t],
                                             start=(t == 0), stop=(t == 3),
                                             perf_mode=DR)
                    src = ps.rearrange("p (t par f) -> p par t f", t=2, par=2)
                    nc.vector.tensor_copy(kts[b][0:64], src[0:64, 0])
                    nc.vector.tensor_copy(kts[b][64:128], src[64:128, 1])
                return f

            def g_q(s):
                def f():
                    ps = psum_proj.tile([128, BLK], f32, tag="p", name=f"pq{s}")
                    if s == 0:
                        for k in range(8):
                            nc.tensor.matmul(ps, wqb_sb[:, k], xbw[0][:, k],
                                             start=(k == 0), stop=(k == 7))
                    else:
                        for t in range(4):
                            nc.tensor.matmul(ps, wq_sb[:, t], x8w[2 * s][:, t],
                                             start=(t == 0), stop=(t == 3),
                                             perf_mode=DR)
                    nc.vector.tensor_copy(qts[s], ps)
                return f

            def g_v(b):
                def f():
                    ps = psum_proj.tile([128, BLK], f32, tag="p", name=f"pv{b}")
                    for t in range(4):
                        nc.tensor.matmul(ps[0:64, :], wv8_sb[:, t], x8w[b][:, t],
                                         start=(t == 0), stop=(t == 3),
                                         perf_mode=DR)
                    vts = stg.tile([128, BLK], bf16, tag="vts", name=f"vts{b}")
                    nc.vector.tensor_copy(vts[0:64, :], ps[0:64, :])
                    tp = psum_proj.tile([128, 4, 256], bf16, tag="p",
                                        name=f"tp{b}")
                    for c in range(4):
                        nc.tensor.transpose(
                            tp[:, c, 0:HEAD], vts[0:64, c * 128:(c + 1) * 128],
                            id_sb[0:64, 0:HEAD])
                    nc.vector.tensor_copy(
                        v18[b][:, :, :, 0:HEAD],
                        tp[:, :, 0:HEAD].rearrange("p (t i) f -> p t i f", t=2))
                return f

            def g_vb(b):
                def f():
                    ps = psum_proj.tile([128, BLK], f32, tag="p", name=f"pvb{b}")
                    for k in range(8):
                        nc.tensor.matmul(ps[0:64, :], wvb_sb[:, k], xbw[b][:, k],
                                         start=(k == 0), stop=(k == 7))
                    vts = stg.tile([128, BLK], bf16, tag="vts", name=f"vtsb{b}")
                    nc.vector.tensor_copy(vts[0:64, :], ps[0:64, :])
                    tp = psum_proj.tile([128, 4, 256], bf16, tag="p",
                                        name=f"tpb{b}")
                    for c in range(4):
                        nc.tensor.transpose(
                            tp[:, c, 0:HEAD], vts[0:64, c * 128:(c + 1) * 128],
                            id_sb[0:64, 0:HEAD])
                    nc.vector.tensor_copy(v1b[b][:, :, 0:HEAD], tp[:, :, 0:HEAD])
                return f

            slot_state = {}

            def g_pair(s, b, t, last):
                def f():
                    diag1 = (b == 2 * s and t == 1)
                    lo = 256 if diag1 else 0
                    sp = psum_s.tile([128, 2, BLK], f32, tag="s",
                                     name=f"s{s}b{b}t{t}")
                    _is = []
                    for i in range(2):
                        _is.append(nc.tensor.matmul(
                            sp[:, i, lo:BLK],
                            kts[b][64 * i:64 * i + 64, t],
                            qts[s][64 * i:64 * i + 64, lo:BLK],
                            start=True, stop=True))
                    _i1, _i2 = _is
                    DBG.append((f"scoreA_s{s}b{b}t{t}", _i1.concise()[:40]))
                    DBG.append((f"scoreB_s{s}b{b}t{t}", _i2.concise()[:40]))
                    bias = pb_sb[:, s:s + 1] if b == 2 * s + 1 else 0.0
                    dt = bf16 if s == 0 else f8
                    tag = "ptb" if s == 0 else "pt"
                    pt = ptp.tile([128, 2, BLK], dt, tag=tag,
                                  name=f"pt{s}b{b}t{t}")
                    _ie = nc.scalar.activation(pt[:, :, lo:BLK], sp[:, :, lo:BLK],
                                         Exp, bias=bias, scale=0.125)
                    DBG.append((f"exp_s{s}b{b}t{t}", _ie.concise()[:40]))
                    if b == 2 * s:
                        c0 = 2 * t * 128
                        nc.vector.tensor_mul(
                            pt[:, 0, c0:c0 + 128], pt[:, 0, c0:c0 + 128], tri_sb)
                        nc.vector.tensor_mul(
                            pt[:, 1, c0 + 128:c0 + 256],
                            pt[:, 1, c0 + 128:c0 + 256], tri_sb)
                        nc.vector.memset(pt[:, 1, lo:lo + 128], 0.0)
                    oacc = slot_state[s]
                    first = slot_state.get((s, "first"), True)
                    if s == 0:
                        for i in range(2):
                            nc.tensor.matmul(
                                oacc[0:HEAD + 2, lo:BLK], v1b[b][:, 2 * t + i],
                                pt[:, i, lo:BLK],
                                start=first and i == 0, stop=last and i == 1)
                    else:
                        nc.tensor.matmul(
                            oacc[0:80, lo:BLK], v18[b][:, t], pt[:, :, lo:BLK],
                            start=first, stop=last, perf_mode=DR)
                    slot_state[(s, "first")] = False
                    if last:
                        ost = stg.tile([128, BLK], f32, tag="ost", name=f"ost{s}")
                        nc.vector.tensor_copy(ost[0:HEAD + 1, :],
                                              oacc[0:HEAD + 1, :])
                        nc.sync.dma_start(out=out[s], in_=ost[0:HEAD + 1, :])
                return f

            # ---- interleaved emission schedule ----
            # Waves arrive in order [0,2,1,3,4,6,5,7] so attention work
            # (slot s needs Q at wave 2s, keys at waves <= 2s+1) becomes
            # available smoothly; at most two slots accumulate at a time
            # (psum_o bufs=2).  Pairs are held one wave and interleaved with
            # the next wave's proj groups so the PE FIFO never drains while
            # ACT paces the exps.
            ORDER = [0, 2, 1, 3, 4, 6, 5, 7]
            # per-slot remaining pair lists (emission order within slot)
            npairs = {s: 2 * (2 * s + 2) for s in range(4)}
            emitted = {s: 0 for s in range(4)}
            arrived = []
            prev_ap = []
            for w in ORDER:
                if w == 0:
                    # fast path to the first exps: slot 0's diagonal pairs
                    # only need K0/Q0 and the bf16 V, not the fp8 V group.
                    g_k(0)()
                    g_q(0)()
                    g_vb(0)()
                    slot_state[0] = psum_o.tile([128, BLK], f32, tag="o",
                                                name="o0")
                    slot_state[(0, "first")] = True
                    slot_state[(0, 0)] = True
                    for t in range(2):
                        emitted[0] += 1
                        g_pair(0, 0, t, last=False)()
                    g_v(0)()
                    arrived.append(0)
                    continue
                pg = [g_k(w)]
                if w % 2 == 0:
                    pg.append(g_q(w // 2))
                pg.append(g_v(w))
                if w < 2:
                    pg.append(g_vb(w))
                while pg or prev_ap:
                    if pg:
                        pg.pop(0)()
                    if prev_ap:
                        prev_ap.pop(0)()
                arrived.append(w)
                for s in range(4):
                    if 2 * s not in arrived:
                        continue
                    if s not in slot_state:
                        slot_state[s] = psum_o.tile([128, BLK], f32, tag="o",
                                                    name=f"o{s}")
                        slot_state[(s, "first")] = True
                    for b in range(2 * s + 2):
                        if b not in arrived:
                            continue
                        if (s, b) in slot_state:
                            continue
                        slot_state[(s, b)] = True
                        for t in range(2):
                            emitted[s] += 1
                            prev_ap.append(
                                g_pair(s, b, t, last=(emitted[s] == npairs[s])))
            for f in prev_ap:
                f()

    nc.compile()
    return nc


def _host_inputs(embedded, Wq, Wk, Wv):
    """Per-core input maps (host does layout only: permute/pack/cast)."""
    bf = ml_dtypes.bfloat16
    f8 = ml_dtypes.float8_e4m3
    emb = np.asarray(embedded, dtype=np.float32)
    wq = np.asarray(Wq, dtype=np.float32)
    wk = np.asarray(Wk, dtype=np.float32)
    wv = np.asarray(Wv, dtype=np.float32)

    def packw(w, dup, dt):  # [1024, 64] -> [128, 4, 2, 64|128]
        r = w.reshape(4, 128, 2, HEAD).transpose(1, 0, 2, 3)
        if dup:
            r = np.concatenate([r, r], axis=3)
        return np.ascontiguousarray(r.astype(dt))

    wq2 = packw(wq, True, f8)
    wk2 = packw(wk, True, f8)
    wvf8 = packw(wv, False, f8)
    wvbf = np.ascontiguousarray(
        wv.reshape(8, 128, HEAD).transpose(1, 0, 2).astype(bf))

    def packwb(w):  # [1024, 64] -> [128, 8, 128] dup
        r = w.reshape(8, 128, HEAD).transpose(1, 0, 2)
        return np.ascontiguousarray(np.concatenate([r, r], axis=2).astype(bf))

    wqbf = packwb(wq)
    wkbf = packwb(wk)

    p = np.arange(128)[:, None]
    f = np.arange(128)[None, :]
    trid = np.ascontiguousarray((p <= f).astype(bf))
    idend = np.ascontiguousarray(np.eye(128).astype(bf))

    in_maps = []
    for b in range(B):
        for role in range(2):
            own, oth = OWN[role], OTHER[role]
            perm = [None] * 8
            for i in range(4):
                perm[2 * i] = own[i]
                perm[2 * i + 1] = oth[i]
            xT = emb[b].T  # [E, T]
            xp = np.concatenate(
                [xT[:, j * BLK:(j + 1) * BLK] for j in perm], axis=1)
            xf8 = np.ascontiguousarray(
                xp.reshape(4, 128, 2, 8, BLK)
                .transpose(1, 3, 0, 2, 4).astype(f8))
            xbf01 = np.ascontiguousarray(
                xp[:, 0:2 * BLK].reshape(8, 128, 2, BLK)
                .transpose(1, 2, 0, 3).astype(bf))
            pbv = np.where(np.array([oth[i] < own[i] for i in range(4)]),
                           0.0, -1e5).astype(np.float32)
            padb = np.ascontiguousarray(
                np.broadcast_to(pbv[None, :], (128, 4)).astype(np.float32))
            in_maps.append({
                "xf8": xf8, "xbf01": xbf01, "wq2": wq2, "wk2": wk2,
                "wvf8": wvf8, "wvbf": wvbf, "trid": trid, "padb": padb,
                "iden": idend, "wqbf": wqbf, "wkbf": wkbf,
            })
    return in_maps


def _run(nc, in_maps, trace=False):
    from concourse.bass_utils import run_bass_kernel_spmd
    return run_bass_kernel_spmd(nc, in_maps, list(range(NCORES)), trace=trace)


def _assemble(results):
    head = np.empty((B, T, HEAD), dtype=np.float32)
    for core, r in enumerate(results):
        b, role = divmod(core, 2)
        o = np.asarray(r["out"])  # [4, 65, 512]
        num = o[:, 0:HEAD, :]
        den = o[:, HEAD:HEAD + 1, :]
        h = (num / den).transpose(0, 2, 1)  # [4, 512, 64]
        for s in range(4):
            j = OWN[role][s]
            head[b, j * BLK:(j + 1) * BLK, :] = h[s]
    return np.tile(head, (1, 1, NH))


def kernel(embedded, Wq, Wk, Wv, num_heads):
    num_heads = int(num_heads)
    assert num_heads == NH

    if "nc" not in _prog_cache:
        _prog_cache["nc"] = _build_program()
    nc = _prog_cache["nc"]

    in_maps = _host_inputs(embedded, Wq, Wk, Wv)
    res = _run(nc, in_maps, trace=bool(int(os.environ.get("KERNEL_TRACE", "0"))))
    _prog_cache["last_result"] = res
    return _assemble(res.results)


# revision 25
# speedup vs baseline: 1.4276x; 1.1084x over previous
"""Causal single-head attention (shared-weight multi-head), 8-core Trainium2 Bass kernel.

Problem: embedded [4, 4096, 1024] f32, Wq/Wk/Wv [1024, 64] f32.
  q/k/v = embedded @ W*;  S = q k^T / 8 (causal);  P = softmax(S);  head = P v
  output = tile(head, 16) -> [4, 4096, 1024] f32.

Sharding: 8 cores = 4 batches x 2 roles; role 0 owns row blocks {0,3,4,7},
role 1 owns {1,2,5,6} (equal causal work). Host permutes the 8 token blocks
per core to positions (own0, oth0, own1, oth1, ...): slot s (= own block s,
rows at position 2s) attends key positions {0..2s+1}; diagonal at position
2s (static tri mask); position 2s+1 is valid-or-padding, zeroed via the exp
bias operand (role-specific data, -1e5 when padding).

Per-core pipeline (engines overlapped; proj and attention interleaved at
mm-group granularity so the PE FIFO never drains behind ACT):
  - x streamed per 512-token block: fp8e4 packed [128,4,2,T] for Q/K proj
    (DoubleRow matmuls, 2x), bf16 for V proj of positions 0-1 only.
  - Q^T emitted duplicated on both partition halves ([wq|wq] stationary);
    K^T emitted split even/odd chunks across partition halves -> the two
    128-key score chunks of a pair run as concurrent row-tiled matmuls
    (contraction 64 uses half the PE rows each).
  - V^T (fp8 DR + bf16 for pos 0/1) -> PE transpose (identity matmul)
    -> V1 = [V | 1] stationaries (fp8 pairs for DoubleRow PV, bf16 slot 0).
  - scores pair -> one ACT exp per pair ([128,2,512] PSUM->SBUF, scale=1/8,
    bias=padbias) -> pt fp8/bf16 -> PV accumulates out^T[0:65,512] in PSUM
    (row 64 = softmax denominator via the ones column).
  - out^T staged to SBUF, DMA'd f32; host divides num/denom, un-permutes
    rows, tiles x16. All x waves stream upfront on the SP HWDGE ring in
    wave-arrival order [0,2,1,3,4,6,5,7]; weight loads ride the ACT ring
    critical-first (each DMA carries ~1-2us fixed completion latency).
"""

import os
import numpy as np
import ml_dtypes

B, T, E, HEAD, NH = 4, 4096, 1024, 64, 16
BLK = 512
NCORES = 8
OWN = {0: [0, 3, 4, 7], 1: [1, 2, 5, 6]}
OTHER = {0: [1, 2, 5, 6], 1: [0, 3, 4, 7]}

_prog_cache = {}
DBG = []


def _build_program(reps=None):
    import concourse.bass as bass
    import concourse.mybir as mybir
    import concourse.tile as tile
    from concourse import bacc

    f32 = mybir.dt.float32
    bf16 = mybir.dt.bfloat16
    f8 = mybir.dt.float8e4
    DR = mybir.MatmulPerfMode.DoubleRow
    Exp = mybir.ActivationFunctionType.Exp

    nc = bacc.Bacc("TRN2", target_bir_lowering=False, debug=False, num_devices=NCORES)

    xf8 = nc.dram_tensor("xf8", [128, 8, 4, 2, BLK], f8, kind="ExternalInput").ap()
    xbf01 = nc.dram_tensor("xbf01", [128, 2, 8, BLK], bf16, kind="ExternalInput").ap()
    wq2 = nc.dram_tensor("wq2", [128, 4, 2, 128], f8, kind="ExternalInput").ap()
    wk2 = nc.dram_tensor("wk2", [128, 4, 2, 128], f8, kind="ExternalInput").ap()
    wvf8 = nc.dram_tensor("wvf8", [128, 4, 2, HEAD], f8, kind="ExternalInput").ap()
    wvbf = nc.dram_tensor("wvbf", [128, 8, HEAD], bf16, kind="ExternalInput").ap()
    wqbf = nc.dram_tensor("wqbf", [128, 8, 128], bf16, kind="ExternalInput").ap()
    wkbf = nc.dram_tensor("wkbf", [128, 8, 128], bf16, kind="ExternalInput").ap()
    trid = nc.dram_tensor("trid", [128, 128], bf16, kind="ExternalInput").ap()
    padb = nc.dram_tensor("padb", [128, 4], f32, kind="ExternalInput").ap()
    iden = nc.dram_tensor("iden", [128, 128], bf16, kind="ExternalInput").ap()
    out = nc.dram_tensor("out", [4, HEAD + 1, BLK], f32, kind="ExternalOutput").ap()

    import contextlib

    with tile.TileContext(nc) as tc:
        loop_ctx = tc.For_i(0, reps, 1) if reps else contextlib.nullcontext()
        with (
            loop_ctx,
            tc.tile_pool(name="singles", bufs=1) as singles,
            tc.tile_pool(name="stg", bufs=2) as stg,
            tc.tile_pool(name="ptp", bufs=6) as ptp,
            tc.tile_pool(name="psum_proj", bufs=2, space="PSUM") as psum_proj,
            tc.tile_pool(name="psum_s", bufs=2, space="PSUM") as psum_s,
            tc.tile_pool(name="psum_o", bufs=2, space="PSUM") as psum_o,
        ):
            # ---- persistent SBUF tiles (per-block where written in waves) ----
            x8w = [singles.tile([128, 4, 2, BLK], f8, name=f"x8w{b}")
                   for b in range(8)]
            xbw = [singles.tile([128, 8, BLK], bf16, name=f"xbw{b}")
                   for b in range(2)]
            wq_sb = singles.tile([128, 4, 2, 128], f8)
            wk_sb = singles.tile([128, 4, 2, 128], f8)
            wv8_sb = singles.tile([128, 4, 2, HEAD], f8)
            wvb_sb = singles.tile([128, 8, HEAD], bf16)
            wqb_sb = singles.tile([128, 8, 128], bf16)
            wkb_sb = singles.tile([128, 8, 128], bf16)
            tri_sb = singles.tile([128, 128], bf16)
            pb_sb = singles.tile([128, 4], f32)
            id_sb = singles.tile([128, 128], bf16)
            qts = [singles.tile([128, BLK], bf16, name=f"qt{s}") for s in range(4)]
            kts = [singles.tile([128, 2, 128], bf16, name=f"kt{b}")
                   for b in range(8)]
            v18 = [singles.tile([128, 2, 2, 80], f8, name=f"v18_{b}")
                   for b in range(8)]
            v1b = [singles.tile([128, 4, HEAD + 2], bf16, name=f"v1b{b}")
                   for b in range(2)]
            dummy = singles.tile([128, 1], f32)

            # ---- weights / masks / first waves; ACT table preload ----
            # critical-first: padb gates the ACT table preload; wkb/wqb
            # gate wave-0 projections; fp8 weights are needed a wave later.
            nc.scalar.dma_start(out=pb_sb, in_=padb)
            nc.scalar.dma_start(out=wkb_sb, in_=wkbf)
            nc.scalar.dma_start(out=wqb_sb, in_=wqbf)
            nc.scalar.dma_start(out=wvb_sb, in_=wvbf)
            nc.scalar.dma_start(out=wv8_sb, in_=wvf8)
            nc.scalar.dma_start(out=tri_sb, in_=trid)
            nc.scalar.dma_start(out=id_sb, in_=iden)
            nc.scalar.dma_start(out=wk_sb, in_=wk2)
            nc.scalar.dma_start(out=wq_sb, in_=wq2)
            # all x waves stream upfront in wave-arrival order (no deps)
            nc.sync.dma_start(out=xbw[0], in_=xbf01[:, 0])
            nc.sync.dma_start(out=x8w[0], in_=xf8[:, 0])
            nc.sync.dma_start(out=x8w[2], in_=xf8[:, 2])
            nc.sync.dma_start(out=xbw[1], in_=xbf01[:, 1])
            nc.sync.dma_start(out=x8w[1], in_=xf8[:, 1])
            for _b in (3, 4, 6, 5, 7):
                nc.sync.dma_start(out=x8w[_b], in_=xf8[:, _b])
            nc.scalar.activation(dummy, pb_sb[:, 0:1], Exp)  # table load early
            for b in range(8):
                nc.vector.memset(v18[b][:, :, :, HEAD:HEAD + 1], 1.0)
                nc.vector.memset(v18[b][:, :, :, HEAD + 1:], 0.0)
            for b in range(2):
                nc.vector.memset(v1b[b][:, :, HEAD:HEAD + 1], 1.0)
                nc.vector.memset(v1b[b][:, :, HEAD + 1:], 0.0)

            # ---- emission thunks: proj groups / attention pairs ----
            def g_k(b):
                def f():
                    ps = psum_proj.tile([128, BLK], f32, tag="p", name=f"pk{b}")
                    if b == 0:
                        for k in range(8):
                            nc.tensor.matmul(ps, wkb_sb[:, k], xbw[0][:, k],
                                             start=(k == 0), stop=(k == 7))
                    else:
                        for t in range(4):
                            nc.tensor.matmul(ps, wk_sb[:, t], x8w[b][:, t],
                                             start=(t == 0), stop=(t == 3),
                                             perf_mode=DR)
                    src = ps.rearrange("p (t par f) -> p par t f", t=2, par=2)
                    nc.vector.tensor_copy(kts[b][0:64], src[0:64, 0])
                    nc.vector.tensor_copy(kts[b][64:128], src[64:128, 1])
                return f

            def g_q(s):
                def f():
                    ps = psum_proj.tile([128, BLK], f32, tag="p", name=f"pq{s}")
                    if s == 0:
                        for k in range(8):
                            nc.tensor.matmul(ps, wqb_sb[:, k], xbw[0][:, k],
                                             start=(k == 0), stop=(k == 7))
                    else:
                        for t in range(4):
                            nc.tensor.matmul(ps, wq_sb[:, t], x8w[2 * s][:, t],
                                             start=(t == 0), stop=(t == 3),
                                             perf_mode=DR)
                    nc.vector.tensor_copy(qts[s], ps)
                return f

            def g_v(b):
                def f():
                    ps = psum_proj.tile([128, BLK], f32, tag="p", name=f"pv{b}")
                    for t in range(4):
                        nc.tensor.matmul(ps[0:64, :], wv8_sb[:, t], x8w[b][:, t],
                                         start=(t == 0), stop=(t == 3),
                                         perf_mode=DR)
                    vts = stg.tile([128, BLK], bf16, tag="vts", name=f"vts{b}")
                    nc.vector.tensor_copy(vts[0:64, :], ps[0:64, :])
                    tp = psum_proj.tile([128, 4, 256], bf16, tag="p",
                                        name=f"tp{b}")
                    for c in range(4):
                        nc.tensor.transpose(
                            tp[:, c, 0:HEAD], vts[0:64, c * 128:(c + 1) * 128],
                            id_sb[0:64, 0:HEAD])
                    nc.vector.tensor_copy(
                        v18[b][:, :, :, 0:HEAD],
                        tp[:, :, 0:HEAD].rearrange("p (t i) f -> p t i f", t=2))
                return f

            def g_vb(b):
                def f():
                    ps = psum_proj.tile([128, BLK], f32, tag="p", name=f"pvb{b}")
                    for k in range(8):
                        nc.tensor.matmul(ps[0:64, :], wvb_sb[:, k], xbw[b][:, k],
                                         start=(k == 0), stop=(k == 7))
                    vts = stg.tile([128, BLK], bf16, tag="vts", name=f"vtsb{b}")
                    nc.vector.tensor_copy(vts[0:64, :], ps[0:64, :])
                    tp = psum_proj.tile([128, 4, 256], bf16, tag="p",
                                        name=f"tpb{b}")
                    for c in range(4):
                        nc.tensor.transpose(
                            tp[:, c, 0:HEAD], vts[0:64, c * 128:(c + 1) * 128],
                            id_sb[0:64, 0:HEAD])
                    nc.vector.tensor_copy(v1b[b][:, :, 0:HEAD], tp[:, :, 0:HEAD])
                return f

            slot_state = {}

            def g_pair(s, b, t, last, mode="all"):
                def f():
                    diag1 = (b == 2 * s and t == 1)
                    lo = 256 if diag1 else 0
                    if mode in ("all", "scores"):
                        sp = psum_s.tile([128, 2, BLK], f32, tag="s",
                                         name=f"s{s}b{b}t{t}")
                        for i in range(2):
                            nc.tensor.matmul(
                                sp[:, i, lo:BLK],
                                kts[b][64 * i:64 * i + 64, t],
                                qts[s][64 * i:64 * i + 64, lo:BLK],
                                start=True, stop=True)
                        bias = pb_sb[:, s:s + 1] if b == 2 * s + 1 else 0.0
                        dt = bf16 if s == 0 else f8
                        tag = "ptb" if s == 0 else "pt"
                        pt = ptp.tile([128, 2, BLK], dt, tag=tag,
                                      name=f"pt{s}b{b}t{t}")
                        nc.scalar.activation(pt[:, :, lo:BLK], sp[:, :, lo:BLK],
                                             Exp, bias=bias, scale=0.125)
                        if b == 2 * s:
                            c0 = 2 * t * 128
                            nc.vector.tensor_mul(
                                pt[:, 0, c0:c0 + 128], pt[:, 0, c0:c0 + 128],
                                tri_sb)
                            nc.vector.tensor_mul(
                                pt[:, 1, c0 + 128:c0 + 256],
                                pt[:, 1, c0 + 128:c0 + 256], tri_sb)
                            nc.vector.memset(pt[:, 1, lo:lo + 128], 0.0)
                        slot_state[(s, b, t, "pt")] = pt
                    if mode == "scores":
                        return
                    pt = slot_state.pop((s, b, t, "pt"))
                    oacc = slot_state[s]
                    first = slot_state.get((s, "first"), True)
                    if s == 0:
                        for i in range(2):
                            nc.tensor.matmul(
                                oacc[0:HEAD + 2, lo:BLK], v1b[b][:, 2 * t + i],
                                pt[:, i, lo:BLK],
                                start=first and i == 0, stop=last and i == 1)
                    else:
                        nc.tensor.matmul(
                            oacc[0:80, lo:BLK], v18[b][:, t], pt[:, :, lo:BLK],
                            start=first, stop=last, perf_mode=DR)
                    slot_state[(s, "first")] = False
                    if last:
                        ost = stg.tile([128, BLK], f32, tag="ost", name=f"ost{s}")
                        nc.vector.tensor_copy(ost[0:HEAD + 1, :],
                                              oacc[0:HEAD + 1, :])
                        nc.gpsimd.dma_start(out=out[s], in_=ost[0:HEAD + 1, :])
                return f

            # ---- interleaved emission schedule ----
            # Waves arrive in order [0,2,1,3,4,6,5,7] so attention work
            # (slot s needs Q at wave 2s, keys at waves <= 2s+1) becomes
            # available smoothly; at most two slots accumulate at a time
            # (psum_o bufs=2).  Pairs are held one wave and interleaved with
            # the next wave's proj groups so the PE FIFO never drains while
            # ACT paces the exps.
            ORDER = [0, 2, 1, 3, 4, 6, 5, 7]
            # per-slot remaining pair lists (emission order within slot)
            npairs = {s: 2 * (2 * s + 2) for s in range(4)}
            emitted = {s: 0 for s in range(4)}
            arrived = []
            prev_ap = []
            for w in ORDER:
                if w == 0:
                    # fast path to the first exps: slot 0's diagonal scores
                    # need only K0/Q0; their PV waits for the bf16 V group.
                    g_k(0)()
                    g_q(0)()
                    slot_state[0] = psum_o.tile([128, BLK], f32, tag="o",
                                                name="o0")
                    slot_state[(0, "first")] = True
                    slot_state[(0, 0)] = True
                    for t in range(2):
                        emitted[0] += 1
                        g_pair(0, 0, t, last=False, mode="scores")()
                    g_vb(0)()
                    for t in range(2):
                        g_pair(0, 0, t, last=False, mode="pv")()
                    g_v(0)()
                    arrived.append(0)
                    continue
                pg = [g_k(w)]
                if w % 2 == 0:
                    pg.append(g_q(w // 2))
                pg.append(g_v(w))
                if w < 2:
                    pg.append(g_vb(w))
                while pg or prev_ap:
                    if pg:
                        pg.pop(0)()
                    if prev_ap:
                        prev_ap.pop(0)()
                arrived.append(w)
                for s in range(4):
                    if 2 * s not in arrived:
                        continue
                    if s not in slot_state:
                        slot_state[s] = psum_o.tile([128, BLK], f32, tag="o",
                                                    name=f"o{s}")
                        slot_state[(s, "first")] = True
                    for b in range(2 * s + 2):
                        if b not in arrived:
                            continue
                        if (s, b) in slot_state:
                            continue
                        slot_state[(s, b)] = True
                        for t in range(2):
                            emitted[s] += 1
                            prev_ap.append(
                                g_pair(s, b, t, last=(emitted[s] == npairs[s])))
            for f in prev_ap:
                f()

    nc.compile()
    return nc


def _host_inputs(embedded, Wq, Wk, Wv):
    """Per-core input maps (host does layout only: permute/pack/cast)."""
    bf = ml_dtypes.bfloat16
    f8 = ml_dtypes.float8_e4m3
    emb = np.asarray(embedded, dtype=np.float32)
    wq = np.asarray(Wq, dtype=np.float32)
    wk = np.asarray(Wk, dtype=np.float32)
    wv = np.asarray(Wv, dtype=np.float32)

    def packw(w, dup, dt):  # [1024, 64] -> [128, 4, 2, 64|128]
        r = w.reshape(4, 128, 2, HEAD).transpose(1, 0, 2, 3)
        if dup:
            r = np.concatenate([r, r], axis=3)
        return np.ascontiguousarray(r.astype(dt))

    wq2 = packw(wq, True, f8)
    wk2 = packw(wk, True, f8)
    wvf8 = packw(wv, False, f8)
    wvbf = np.ascontiguousarray(
        wv.reshape(8, 128, HEAD).transpose(1, 0, 2).astype(bf))

    def packwb(w):  # [1024, 64] -> [128, 8, 128] dup
        r = w.reshape(8, 128, HEAD).transpose(1, 0, 2)
        return np.ascontiguousarray(np.concatenate([r, r], axis=2).astype(bf))

    wqbf = packwb(wq)
    wkbf = packwb(wk)

    p = np.arange(128)[:, None]
    f = np.arange(128)[None, :]
    trid = np.ascontiguousarray((p <= f).astype(bf))
    idend = np.ascontiguousarray(np.eye(128).astype(bf))

    in_maps = []
    for b in range(B):
        for role in range(2):
            own, oth = OWN[role], OTHER[role]
            perm = [None] * 8
            for i in range(4):
                perm[2 * i] = own[i]
                perm[2 * i + 1] = oth[i]
            xT = emb[b].T  # [E, T]
            xp = np.concatenate(
                [xT[:, j * BLK:(j + 1) * BLK] for j in perm], axis=1)
            xf8 = np.ascontiguousarray(
                xp.reshape(4, 128, 2, 8, BLK)
                .transpose(1, 3, 0, 2, 4).astype(f8))
            xbf01 = np.ascontiguousarray(
                xp[:, 0:2 * BLK].reshape(8, 128, 2, BLK)
                .transpose(1, 2, 0, 3).astype(bf))
            pbv = np.where(np.array([oth[i] < own[i] for i in range(4)]),
                           0.0, -1e5).astype(np.float32)
            padb = np.ascontiguousarray(
                np.broadcast_to(pbv[None, :], (128, 4)).astype(np.float32))
            in_maps.append({
                "xf8": xf8, "xbf01": xbf01, "wq2": wq2, "wk2": wk2,
                "wvf8": wvf8, "wvbf": wvbf, "trid": trid, "padb": padb,
                "iden": idend, "wqbf": wqbf, "wkbf": wkbf,
            })
    return in_maps


def _run(nc, in_maps, trace=False):
    from concourse.bass_utils import run_bass_kernel_spmd
    return run_bass_kernel_spmd(nc, in_maps, list(range(NCORES)), trace=trace)


def _assemble(results):
    head = np.empty((B, T, HEAD), dtype=np.float32)
    for core, r in enumerate(results):
        b, role = divmod(core, 2)
        o = np.asarray(r["out"])  # [4, 65, 512]
        num = o[:, 0:HEAD, :]
        den = o[:, HEAD:HEAD + 1, :]
        h = (num / den).transpose(0, 2, 1)  # [4, 512, 64]
        for s in range(4):
            j = OWN[role][s]
            head[b, j * BLK:(j + 1) * BLK, :] = h[s]
    return np.tile(head, (1, 1, NH))


def kernel(embedded, Wq, Wk, Wv, num_heads):
    num_heads = int(num_heads)
    assert num_heads == NH

    if "nc" not in _prog_cache:
        _prog_cache["nc"] = _build_program()
    nc = _prog_cache["nc"]

    in_maps = _host_inputs(embedded, Wq, Wk, Wv)
    res = _run(nc, in_maps, trace=bool(int(os.environ.get("KERNEL_TRACE", "0"))))
    _prog_cache["last_result"] = res
    return _assemble(res.results)
